# revision 46
# baseline (speedup 1.0000x reference)
"""GQA attention (RoPE, causal) on 8 Trainium2 NeuronCores, tensor-parallel
over heads: each core owns 4 query heads + 1 kv head, computes its slice of
qkv, attention, and a partial output projection; the host sums the 8 partial
projections.

All matmuls and the bulk dataflow run in bf16 (fp32 PSUM accumulation), so
x/weights stream at half the HBM bytes and DVE elementwise ops hit the
2x/4x 16-bit modes. Scores are computed transposed ([st, sq]) with
64-partition contraction (no zero-padded K operands), so the softmax
denominator comes out of the attn@V matmul itself via a ones-column
appended to V (M=65), and exp needs no max-subtraction (logits are bounded;
fp32 PSUM can't overflow). Normalization uses reciprocal_approx_fast plus a
GpSimd partition_broadcast (no PE broadcast matmuls). The projection of
chunk c-1 is interleaved into the flash loop of chunk c so the PE has work
while the ScalarE runs exp. Phase A (qkv+RoPE) is software-pipelined in
four 512-column chunks so PSUM evacuation overlaps the next chunk's
matmuls.
"""

import numpy as np

HIDDEN = 2048
HEAD_DIM = 64
N_HEADS = 32
N_KV_HEADS = 8
S = 2048
N_CORES = 8
HPC = N_HEADS // N_CORES          # q heads per core = 4
D = HEAD_DIM
KT = HIDDEN // 128                # 16 contraction tiles for qkv
ST = S // 128                     # 16 seq tiles of 128
NC4 = S // 512                    # 4 seq chunks of 512
OSH = HPC * D + 2 * D             # 384 rows in the per-core qkv weight shard

_CACHE = {}


def _split_excess_waits(nc, mybir):
    """The staged walrus accepts at most one sync wait per instruction (two
    on EventSemaphore); Tile attaches more. Hoist extras onto same-engine
    NoOps inserted just before the instruction — engine program order then
    preserves the wait semantics."""
    for func in nc.m.functions:
        for block in func.blocks:
            new_insts = []
            for inst in block.instructions:
                si = inst.sync_info
                waits = list(si.on_wait) if si is not None and si.on_wait else []
                cap = 2 if isinstance(inst, mybir.InstEventSemaphore) else 1
                if len(waits) > cap:
                    si.on_wait = waits[:cap]
                    for j, w in enumerate(waits[cap:]):
                        nop = mybir.InstNoOp(
                            name=f"{inst.name}-ws{j}",
                            ins=[], outs=[], engine=inst.engine,
                        )
                        nop.sync_info = mybir.SyncInfo(on_wait=[w], on_update=[])
                        new_insts.append(nop)
                new_insts.append(inst)
            block.instructions = new_insts


def _build():
    import concourse.bass as bass
    import concourse.tile as tile
    from concourse import mybir

    f32 = mybir.dt.float32
    f32r = mybir.dt.float32r
    bf16 = mybir.dt.bfloat16

    nc = bass.Bass("TRN2", target_bir_lowering=False, debug=False,
                   num_devices=N_CORES)

    xT_d = nc.dram_tensor("xT", [HIDDEN, S], bf16, kind="ExternalInput")
    wq_d = nc.dram_tensor("wqkvT", [HIDDEN, OSH], bf16, kind="ExternalInput")
    wo_d = nc.dram_tensor("woutT", [2 * 128, HIDDEN], bf16, kind="ExternalInput")
    c_d = nc.dram_tensor("ctile", [128, S], bf16, kind="ExternalInput")
    s_d = nc.dram_tensor("stile", [128, S], bf16, kind="ExternalInput")
    rt_d = nc.dram_tensor("rotT", [128, 128], bf16, kind="ExternalInput")
    id_d = nc.dram_tensor("ident", [D, D], bf16, kind="ExternalInput")
    on_d = nc.dram_tensor("ones", [1, D], bf16, kind="ExternalInput")
    mk_d = nc.dram_tensor("masks", [4, 128, 512], bf16, kind="ExternalInput")
    out_d = nc.dram_tensor("out", [HIDDEN, S], bf16, kind="ExternalOutput")

    xT_p = xT_d.rearrange("(a b p) s -> a p b s", b=2, p=128)
    wq_t = wq_d.rearrange("(t p) o -> t p o", p=128)
    out_p = out_d.rearrange("(a p) s -> p a s", p=128)

    scale = 1.0 / float(np.sqrt(D))

    with tile.TileContext(nc) as tc:
        with (
            nc.allow_low_precision(reason="bf16 dataflow is deliberate"),
            tc.tile_pool(name="wts", bufs=1) as wts,
            tc.tile_pool(name="acts", bufs=1) as acts,
            tc.tile_pool(name="xin", bufs=10) as xin,
            tc.tile_pool(name="psb", bufs=6) as psb,
            tc.tile_pool(name="ev", bufs=2) as evp,
            tc.tile_pool(name="evo", bufs=6) as evo,
        ):
            # ---- persistent loads (gpsimd queue, so the sync queue is
            # free for the xT stream; flash-only tensors loaded later) ----
            # wq[0] rides the sync queue, which comes up ~3us before the
            # gpsimd queue — together with the first x k-tiles loaded as
            # singles (below) the first qkv matmul can start at ~7us
            # cos/sin tiles are 1MB and not needed before ~25us: their
            # triggers sit after 12 of the wq loads on the gpsimd queue so
            # the transfers miss the first chunk's x-stream crunch
            c_sb = wts.tile([128, S], bf16, tag="ct", name="ct")
            s_sb = wts.tile([128, S], bf16, tag="st", name="st")
            wq_sb = []
            for k in range(KT):
                t = wts.tile([128, OSH], bf16, tag=f"wq{k}", name="wq")
                nc.gpsimd.dma_start(t[:], wq_t[k])
                wq_sb.append(t)
                if k == 11:
                    nc.gpsimd.dma_start(c_sb[:], c_d[:])
                    nc.gpsimd.dma_start(s_sb[:], s_d[:])
            # small RoPE constants ride the scalar queue, which is otherwise
            # idle until the first psum evacuation: they land ~8us without
            # contending with the x stream (sync) or wq (gpsimd)
            rt_sb = wts.tile([128, 128], bf16, tag="rt", name="rt")
            nc.scalar.dma_start(rt_sb[:], rt_d[:])
            id_sb = wts.tile([D, D], bf16, tag="id", name="id")
            nc.scalar.dma_start(id_sb[:], id_d[:])
            on_sb = wts.tile([1, D], bf16, tag="on", name="on")
            nc.scalar.dma_start(on_sb[:], on_d[:])
            wo_sb = []
            for i in range(2):
                t = wts.tile([128, HIDDEN], bf16, tag=f"wo{i}", name="wo")
                wo_sb.append(t)
            mk_sb = []
            for j in range(4):
                t = wts.tile([128, 512], bf16, tag=f"mk{j}", name="mk")
                mk_sb.append(t)

            # ---- persistent activations (RoPE applied in place) ----
            qr_sb = [acts.tile([128, S], bf16, tag=f"qr{p}", name=f"qr{p}")
                     for p in range(2)]
            kr_sb = acts.tile([128, S], bf16, tag="kr", name="kr")
            vT_sb = acts.tile([D, S], bf16, tag="vT", name="vT")
            v_sb = acts.tile([128, ST, D + 1], bf16, tag="v", name="v")
            outT = [acts.tile([128, S], bf16, tag=f"oT{p}", name=f"oT{p}")
                    for p in range(2)]

            nc.gpsimd.memset(v_sb[:, :, D:D + 1], 1.0)

            # ---- phases A-C, software-pipelined by 512-col chunks:
            # qkv matmuls of chunk j overlap psum evacuation (ScalarE),
            # RoPE (PE rot-matmul + DVE) and v transpose of chunk j-1 ----
            with (
                tc.tile_pool(name="psA", bufs=6, space="PSUM") as psA,
                tc.tile_pool(name="psB", bufs=1, space="PSUM") as psB,
                tc.tile_pool(name="psC", bufs=1, space="PSUM") as psC,
            ):
                ps_chunks = {}

                def qkv_chunk(ch):
                    sl = slice(ch * 512, (ch + 1) * 512)
                    ps = [psA.tile([128, 512], f32, tag="qkv",
                                   name=f"qkv{o}c{ch}") for o in range(3)]
                    ps_chunks[ch] = ps
                    # two k-tiles per dma_start — the sync sequencer's
                    # per-trigger cost (~0.5us) would otherwise pace the
                    # x stream. Chunk 0 loads singles into SEPARATE tiles
                    # (per-tile dep tracking: a shared tile would make the
                    # first matmul wait for both halves) so the PE starts
                    # on k-tile 0 as soon as it lands.
                    for j in range(KT // 2):
                        if ch == 0:
                            xt = [xin.tile([128, 512], bf16, tag="xt1",
                                           name="xt1") for _ in range(2)]
                            for b in range(2):
                                nc.sync.dma_start(xt[b][:],
                                                  xT_p[j][:, b, sl])
                            rhs = [xt[0][:], xt[1][:]]
                        else:
                            xt = xin.tile([128, 2, 512], bf16, tag="xt",
                                          name="xt")
                            nc.sync.dma_start(xt[:], xT_p[j][:, :, sl])
                            rhs = [xt[:, 0, :], xt[:, 1, :]]
                        for b in range(2):
                            k = 2 * j + b
                            for o in range(3):
                                nc.tensor.matmul(
                                    ps[o][:],
                                    lhsT=wq_sb[k][:, o * 128:(o + 1) * 128],
                                    rhs=rhs[b],
                                    start=(k == 0), stop=(k == KT - 1))

                def finish_chunk(ch):
                    sl = slice(ch * 512, (ch + 1) * 512)
                    ps = ps_chunks.pop(ch)
                    nc.scalar.copy(qr_sb[0][:, sl], ps[0][:])
                    nc.scalar.copy(qr_sb[1][:, sl], ps[1][:])
                    # k duplicated into both partition halves so the flash
                    # score matmuls can contract 64 partitions at base 0
                    # (even heads) or base 64 (odd heads) — matmul requires
                    # lhsT/rhs base partitions to match
                    nc.scalar.copy(kr_sb[0:D, sl], ps[2][0:D, :])
                    nc.scalar.copy(kr_sb[D:128, sl], ps[2][0:D, :])
                    nc.scalar.copy(vT_sb[:, sl], ps[2][D:128, :])
                    # RoPE in place:  t = t*C + (R @ t)*S
                    for src_t in (qr_sb[0], qr_sb[1], kr_sb):
                        p = src_t.shape[0]
                        sw = psB.tile([128, 512], f32, tag="sw", name="sw")
                        nc.tensor.matmul(sw[:p, :], lhsT=rt_sb[:p, :p],
                                         rhs=src_t[:, sl],
                                         start=True, stop=True)
                        m1 = evp.tile([p, 512], bf16, tag="m1", name="m1")
                        nc.vector.tensor_mul(m1[:], src_t[:, sl],
                                             c_sb[:p, sl])
                        m2 = evp.tile([p, 512], bf16, tag="m2", name="m2")
                        nc.vector.tensor_mul(m2[:], sw[:p, :], s_sb[:p, sl])
                        nc.vector.tensor_add(src_t[:, sl], m1[:], m2[:])
                    # v transpose for this chunk's four st tiles, batched
                    # into one psum tile -> one DVE evacuation
                    pv = psC.tile([128, 4 * D], bf16, tag="vt", name="vt")
                    for j in range(4):
                        t = 4 * ch + j
                        nc.tensor.transpose(
                            pv[:, j * D:(j + 1) * D],
                            vT_sb[:, t * 128:(t + 1) * 128],
                            id_sb[:])
                    nc.vector.tensor_copy(
                        v_sb[:, 4 * ch:4 * ch + 4, 0:D],
                        pv[:].rearrange("p (t d) -> p t d", d=D))

                for ch in range(NC4 + 1):
                    if ch < NC4:
                        qkv_chunk(ch)
                    if ch >= 1:
                        finish_chunk(ch - 1)
                    # mask/wo loads ride the scalar queue, positioned so the
                    # triggers only fire after the 3rd/4th chunks' psum
                    # evacuations — their transfers then stay out of the
                    # x stream's HBM window (masks land ~10us before the
                    # first flash head, wo well before the first proj)
                    if ch == NC4 - 1:
                        for j in range(4):
                            nc.scalar.dma_start(mk_sb[j][:], mk_d[j])
                    if ch == NC4:
                        for i in range(2):
                            nc.scalar.dma_start(
                                wo_sb[i][:], wo_d[i * 128:(i + 1) * 128, :])

            # ---- phase D/E: flash attention with interleaved projection ----
            with (
                tc.tile_pool(name="scp", bufs=2, space="PSUM") as scp,
                tc.tile_pool(name="avp", bufs=2, space="PSUM") as avp,
                tc.tile_pool(name="mpp", bufs=2, space="PSUM") as mpp,
            ):
                def flash_head(c, h, uo_sb, l_sb):
                    csl = slice(c * 512, (c + 1) * 512)
                    n_st = 4 * c + 4
                    pair, half = divmod(h, 2)
                    qsl = slice(half * D, (half + 1) * D)
                    av = avp.tile([128, 512], f32, tag="av", name="av")

                    # diagonal tile t = 4c+j: the chunk's first 128j queries
                    # are fully masked — scores/mask/attn@V all run on the
                    # suffix only. The skipped sc prefix holds stale (but
                    # bounded) scores from a recycled psum buffer; their exp
                    # is finite and never read.
                    def toff(t):
                        return max(0, 128 * (t - 4 * c))

                    def av_group(g, pt):
                        for i in range(2):
                            t = 2 * g + i
                            off = toff(t)
                            psl = slice(i * 512 + off, (i + 1) * 512)
                            if t >= 4 * c:
                                nc.vector.tensor_mul(
                                    pt[:, psl], pt[:, psl],
                                    mk_sb[t - 4 * c][:, off:])
                            nc.tensor.matmul(
                                av[:D + 1, off:],
                                lhsT=v_sb[:, t, :],
                                rhs=pt[:, psl],
                                start=(t == 0), stop=(t == n_st - 1),
                                skip_group_check=True)

                    prev = None
                    for g in range(n_st // 2):
                        sc = scp.tile([128, 1024], f32, tag="sc", name="sc")
                        for i in range(2):
                            t = 2 * g + i
                            off = toff(t)
                            nc.tensor.matmul(
                                sc[:, i * 512 + off:(i + 1) * 512],
                                lhsT=kr_sb[qsl, t * 128:(t + 1) * 128],
                                rhs=qr_sb[pair][qsl,
                                                c * 512 + off:(c + 1) * 512],
                                start=True, stop=True)
                        pt = psb.tile([128, 1024], bf16, tag="P", name="P")
                        es = toff(2 * g)
                        nc.scalar.activation(
                            pt[:, es:], sc[:, es:],
                            mybir.ActivationFunctionType.Exp, scale=scale)
                        # emit the PREVIOUS group's attn@V after this group's
                        # scores so the PE never head-of-line blocks on exp
                        if prev is not None:
                            av_group(*prev)
                        prev = (g, pt)
                    av_group(*prev)
                    # stash unnormalized out^T and its denominator row
                    nc.vector.tensor_copy(uo_sb[:], av[0:D, :])
                    nc.vector.tensor_copy(l_sb[32 * h:32 * h + 1, :],
                                          av[D:D + 1, :])

                def norm_head(c, h, uo_sb, rcp):
                    csl = slice(c * 512, (c + 1) * 512)
                    pair, half = divmod(h, 2)
                    qsl = slice(half * D, (half + 1) * D)
                    # broadcast 1/l across the 64 head dims via a ones-column
                    # matmul (f32r rhs -> full-rate), then scale in one mul
                    bc = mpp.tile([128, 512], f32, tag="mp", name="mp")
                    nc.tensor.matmul(bc[:D, :], lhsT=on_sb[:],
                                     rhs=rcp[h][:], start=True, stop=True)
                    nc.vector.tensor_mul(
                        outT[pair][qsl, csl], uo_sb[:], bc[:D, :])

                def proj_group(c, hts, drain=False, queue=None):
                    csl = slice(c * 512, (c + 1) * 512)
                    ev = evo.tile([128, 4, 512], bf16, tag="ev", name="ev")
                    for n, ht in enumerate(hts):
                        pr = mpp.tile([128, 512], f32, tag="mp", name="mp")
                        for i in range(2):
                            nc.tensor.matmul(
                                pr[:],
                                lhsT=wo_sb[i][:, ht * 128:(ht + 1) * 128],
                                rhs=outT[i][:, csl],
                                start=(i == 0), stop=(i == 1))
                        # mid-flash the ScalarE is exp-bound: evacuate on
                        # DVE; in the drain both engines are free, split 2/2
                        if drain and ht % 2 == 1:
                            nc.scalar.copy(ev[:, n, :], pr[:])
                        else:
                            nc.vector.tensor_copy(ev[:, n, :], pr[:])
                    # one dma per 4 row-tiles (the sync sequencer's
                    # per-trigger cost would otherwise pace the drain)
                    (queue or nc.sync).dma_start(
                        out_p[:, hts[0]:hts[0] + 4, csl], ev[:])

                # pipeline: flash(ci) | normalize(ci-1) | project(ci-2) —
                # proj must trail normalization of ALL heads of its chunk.
                cs = [1, 2, 0, 3]
                uo_tiles = {}
                rcps = {}
                l_tiles = {}

                def rcp_chain(c):
                    # 1/l as exp(-ln l) on ScalarE: ln and exp share one act
                    # table, and it offloads the reciprocal from the busier
                    # DVE. Issued after the next chunk's first flash head so
                    # it never delays the boundary exp.
                    l_sb = l_tiles.pop(c)
                    lnl = evp.tile([128, 512], f32, tag="lnl", name="lnl")
                    nc.scalar.activation(
                        lnl[:97, :], l_sb[:97, :],
                        mybir.ActivationFunctionType.Ln)
                    rcp = evp.tile([128, 512], f32, tag="rcp", name="rcp")
                    nc.scalar.activation(
                        rcp[:97, :], lnl[:97, :],
                        mybir.ActivationFunctionType.Exp, scale=-1.0)
                    rows = []
                    for h in range(HPC):
                        rh = evp.tile([1, 512], bf16, tag=f"rch{h}",
                                      name=f"rch{h}")
                        nc.vector.tensor_copy(rh[:],
                                              rcp[32 * h:32 * h + 1, :])
                        rows.append(rh)
                    rcps[c] = rows

                for i in range(NC4):
                    l_sb = evp.tile([128, 512], f32, tag="l", name="l")
                    l_tiles[cs[i]] = l_sb
                    for h in range(HPC):
                        uo = evp.tile([D, 512], bf16, tag=f"uo{h}",
                                      name=f"uo{h}")
                        uo_tiles[(cs[i], h)] = uo
                        flash_head(cs[i], h, uo, l_sb)
                        # h==0: proj first so the rcp chain (ScalarE) has
                        # time to land behind the boundary exps; h>=1: norm
                        # first so the last norm mul is not queued behind
                        # this slot's proj evacuations on the DVE
                        if i >= 2 and h == 0:
                            proj_group(cs[i - 2], range(0, 4))
                        if i >= 1 and h == 0:
                            rcp_chain(cs[i - 1])
                        if i >= 1:
                            norm_head(cs[i - 1], h,
                                      uo_tiles.pop((cs[i - 1], h)),
                                      rcps[cs[i - 1]])
                        if i >= 2 and h >= 1:
                            proj_group(cs[i - 2], range(4 * h, 4 * h + 4))
                    if i == NC4 - 1:
                        # last chunk's reciprocal issued right behind its
                        # final exps so the drain's norms start immediately
                        rcp_chain(cs[i])

                # drain: start with the already-normalized chunk's proj (PE
                # filler that covers the last rcp chain's latency), then the
                # final chunk's norms, then the rest; out-writes alternate
                # between the sync and scalar DMA queues
                c_pen, c_fin = cs[NC4 - 2], cs[NC4 - 1]
                proj_group(c_pen, range(0, 4), drain=True, queue=nc.sync)
                proj_group(c_pen, range(4, 8), drain=True, queue=nc.scalar)
                for h in range(HPC):
                    norm_head(c_fin, h, uo_tiles.pop((c_fin, h)),
                              rcps[c_fin])
                proj_group(c_pen, range(8, 12), drain=True, queue=nc.sync)
                proj_group(c_pen, range(12, 16), drain=True, queue=nc.scalar)
                for g in range(HPC):
                    proj_group(c_fin, range(4 * g, 4 * g + 4), drain=True,
                               queue=(nc.sync if g % 2 == 0 else nc.scalar))

    _split_excess_waits(nc, mybir)
    return nc


def _host_prep(x, cos, sin, w_qkv, w_out):
    import ml_dtypes
    bf = ml_dtypes.bfloat16

    xT = np.ascontiguousarray(x[0].T).astype(bf)                # [H, S]
    cosT = cos.T.astype(np.float32)                             # [64, S]
    sinT = sin.T.astype(np.float32)
    ctile = np.ascontiguousarray(np.concatenate([cosT, cosT], 0)).astype(bf)
    stile = np.ascontiguousarray(np.concatenate([sinT, sinT], 0)).astype(bf)

    # rotate_half as a matrix: rot(q)^T = R @ q^T per 64-block; ship R^T
    r = np.zeros((D, D), dtype=np.float32)
    for i in range(32):
        r[i, 32 + i] = -1.0
        r[32 + i, i] = 1.0
    R = np.zeros((128, 128), dtype=np.float32)
    R[:D, :D] = r
    R[D:, D:] = r
    rotT = np.ascontiguousarray(R.T).astype(bf)

    ident = np.eye(D, dtype=np.float32).astype(bf)

    p = np.arange(128)[:, None]
    f = np.arange(512)[None, :]
    masks = np.stack([(p <= f - 128 * j).astype(bf) for j in range(4)])

    shared = {"xT": xT, "ctile": ctile, "stile": stile, "rotT": rotT,
              "ident": ident, "ones": np.ones((1, D), dtype=bf),
              "masks": masks}

    in_maps = []
    for c in range(N_CORES):
        qrows = w_qkv[4 * c * D:(4 * c + 4) * D]                # [256, H]
        krows = w_qkv[N_HEADS * D + c * D: N_HEADS * D + (c + 1) * D]
        vrows = w_qkv[(N_HEADS + N_KV_HEADS) * D + c * D:
                      (N_HEADS + N_KV_HEADS) * D + (c + 1) * D]
        wsh = np.concatenate([qrows, krows, vrows], 0)          # [384, H]
        wqkvT = np.ascontiguousarray(wsh.T).astype(bf)          # [H, 384]
        wo_cols = w_out[:, 4 * c * D:(4 * c + 4) * D]           # [H, 256]
        woutT = np.ascontiguousarray(wo_cols.T).astype(bf)
        in_maps.append({**shared, "wqkvT": wqkvT, "woutT": woutT})
    return in_maps


def kernel(x, cos, sin, w_qkv, w_out):
    from concourse.bass_utils import run_bass_kernel_spmd

    if "nc" not in _CACHE:
        _CACHE["nc"] = _build()
    nc = _CACHE["nc"]

    in_maps = _host_prep(x, cos, sin, w_qkv, w_out)
    res = run_bass_kernel_spmd(nc, in_maps, list(range(N_CORES)))
    total = np.zeros((HIDDEN, S), dtype=np.float32)
    for r in res.results:
        total += r["out"].astype(np.float32)
    return total.T.reshape(1, S, HIDDEN).copy()


# revision 48
# speedup vs baseline: 1.0071x; 1.0071x over previous
"""GQA attention (RoPE, causal) on 8 Trainium2 NeuronCores, tensor-parallel
over heads: each core owns 4 query heads + 1 kv head, computes its slice of
qkv, attention, and a partial output projection; the host sums the 8 partial
projections (bf16) in fp32.

All matmuls and the bulk dataflow run in bf16 (fp32 PSUM accumulation), so
x/weights stream at half the HBM bytes. Scores are computed transposed
([st, sq]) with 64-partition contraction (K duplicated into both partition
halves so even/odd heads' operands share a base partition), the softmax
denominator comes out of the attn@V matmul itself via a ones-column
appended to V (M=65), and exp needs no max-subtraction (logits are bounded;
fp32 PSUM can't overflow). 1/l is exp(-ln l) on the ScalarE (ln+exp share
one act table), broadcast across head dims by a cheap ones x rcp matmul.
Fully-masked 128-column query prefixes of diagonal-band tiles are trimmed
from the score/mask/attn@V/exp work. The projection of chunk c-2 and
normalization of c-1 are interleaved into the flash loop of chunk c so the
PE has work while the ScalarE runs exp (HAM keeps the clock up only while
the PE stays dense). Phase A (qkv+RoPE+V-transpose) is software-pipelined
in four 512-column chunks; DMA trigger issue is batched and spread over
the sync/gpsimd/scalar queues so neither sequencer pacing nor the x-stream
HBM window stalls the PE.
"""

import numpy as np

HIDDEN = 2048
HEAD_DIM = 64
N_HEADS = 32
N_KV_HEADS = 8
S = 2048
N_CORES = 8
HPC = N_HEADS // N_CORES          # q heads per core = 4
D = HEAD_DIM
KT = HIDDEN // 128                # 16 contraction tiles for qkv
ST = S // 128                     # 16 seq tiles of 128
NC4 = S // 512                    # 4 seq chunks of 512
OSH = HPC * D + 2 * D             # 384 rows in the per-core qkv weight shard

_CACHE = {}


def _split_excess_waits(nc, mybir):
    """The staged walrus accepts at most one sync wait per instruction (two
    on EventSemaphore); Tile attaches more. Hoist extras onto same-engine
    NoOps inserted just before the instruction — engine program order then
    preserves the wait semantics."""
    for func in nc.m.functions:
        for block in func.blocks:
            new_insts = []
            for inst in block.instructions:
                si = inst.sync_info
                waits = list(si.on_wait) if si is not None and si.on_wait else []
                cap = 2 if isinstance(inst, mybir.InstEventSemaphore) else 1
                if len(waits) > cap:
                    si.on_wait = waits[:cap]
                    for j, w in enumerate(waits[cap:]):
                        nop = mybir.InstNoOp(
                            name=f"{inst.name}-ws{j}",
                            ins=[], outs=[], engine=inst.engine,
                        )
                        nop.sync_info = mybir.SyncInfo(on_wait=[w], on_update=[])
                        new_insts.append(nop)
                new_insts.append(inst)
            block.instructions = new_insts


def _build():
    import concourse.bass as bass
    import concourse.tile as tile
    from concourse import mybir

    f32 = mybir.dt.float32
    bf16 = mybir.dt.bfloat16

    nc = bass.Bass("TRN2", target_bir_lowering=False, debug=False,
                   num_devices=N_CORES)

    xT_d = nc.dram_tensor("xT", [HIDDEN, S], bf16, kind="ExternalInput")
    wq_d = nc.dram_tensor("wqkvT", [HIDDEN, OSH], bf16, kind="ExternalInput")
    wo_d = nc.dram_tensor("woutT", [2 * 128, HIDDEN], bf16, kind="ExternalInput")
    c_d = nc.dram_tensor("ctile", [128, S], bf16, kind="ExternalInput")
    s_d = nc.dram_tensor("stile", [128, S], bf16, kind="ExternalInput")
    rt_d = nc.dram_tensor("rotT", [128, 128], bf16, kind="ExternalInput")
    id_d = nc.dram_tensor("ident", [D, D], bf16, kind="ExternalInput")
    on_d = nc.dram_tensor("ones", [1, D], bf16, kind="ExternalInput")
    mk_d = nc.dram_tensor("masks", [4, 128, 512], bf16, kind="ExternalInput")
    out_d = nc.dram_tensor("out", [HIDDEN, S], bf16, kind="ExternalOutput")

    xT_p = xT_d.rearrange("(a b p) s -> a p b s", b=2, p=128)
    wq_t = wq_d.rearrange("(t p) o -> t p o", p=128)
    out_p = out_d.rearrange("(a p) s -> p a s", p=128)

    scale = 1.0 / float(np.sqrt(D))

    with tile.TileContext(nc) as tc:
        with (
            nc.allow_low_precision(reason="bf16 dataflow is deliberate"),
            tc.tile_pool(name="wts", bufs=1) as wts,
            tc.tile_pool(name="acts", bufs=1) as acts,
            tc.tile_pool(name="xin", bufs=10) as xin,
            tc.tile_pool(name="psb", bufs=6) as psb,
            tc.tile_pool(name="ev", bufs=2) as evp,
            tc.tile_pool(name="evo", bufs=6) as evo,
        ):
            # ---- persistent loads (gpsimd queue, so the sync queue is
            # free for the xT stream; flash-only tensors loaded later) ----
            # wq[0] rides the sync queue, which comes up ~3us before the
            # gpsimd queue — together with the first x k-tiles loaded as
            # singles (below) the first qkv matmul can start at ~7us
            # cos/sin tiles are 1MB and not needed before ~25us: their
            # triggers sit after 12 of the wq loads on the gpsimd queue so
            # the transfers miss the first chunk's x-stream crunch
            c_sb = wts.tile([128, S], bf16, tag="ct", name="ct")
            s_sb = wts.tile([128, S], bf16, tag="st", name="st")
            wq_sb = []
            for k in range(KT):
                t = wts.tile([128, OSH], bf16, tag=f"wq{k}", name="wq")
                nc.gpsimd.dma_start(t[:], wq_t[k])
                wq_sb.append(t)
                if k == 11:
                    nc.gpsimd.dma_start(c_sb[:], c_d[:])
                    nc.gpsimd.dma_start(s_sb[:], s_d[:])
            # small RoPE constants ride the scalar queue, which is otherwise
            # idle until the first psum evacuation: they land ~8us without
            # contending with the x stream (sync) or wq (gpsimd)
            rt_sb = wts.tile([128, 128], bf16, tag="rt", name="rt")
            nc.scalar.dma_start(rt_sb[:], rt_d[:])
            id_sb = wts.tile([D, D], bf16, tag="id", name="id")
            nc.scalar.dma_start(id_sb[:], id_d[:])
            on_sb = wts.tile([1, D], bf16, tag="on", name="on")
            nc.scalar.dma_start(on_sb[:], on_d[:])
            wo_sb = []
            for i in range(2):
                t = wts.tile([128, HIDDEN], bf16, tag=f"wo{i}", name="wo")
                wo_sb.append(t)
            mk_sb = []
            for j in range(4):
                t = wts.tile([128, 512], bf16, tag=f"mk{j}", name="mk")
                mk_sb.append(t)

            # ---- persistent activations (RoPE applied in place) ----
            qr_sb = [acts.tile([128, S], bf16, tag=f"qr{p}", name=f"qr{p}")
                     for p in range(2)]
            kr_sb = acts.tile([128, S], bf16, tag="kr", name="kr")
            vT_sb = acts.tile([D, S], bf16, tag="vT", name="vT")
            v_sb = acts.tile([128, ST, D + 1], bf16, tag="v", name="v")
            outT = [acts.tile([128, S], bf16, tag=f"oT{p}", name=f"oT{p}")
                    for p in range(2)]

            nc.gpsimd.memset(v_sb[:, :, D:D + 1], 1.0)

            # ---- phases A-C, software-pipelined by 512-col chunks:
            # qkv matmuls of chunk j overlap psum evacuation (ScalarE),
            # RoPE (PE rot-matmul + DVE) and v transpose of chunk j-1 ----
            with (
                tc.tile_pool(name="psA", bufs=6, space="PSUM") as psA,
                tc.tile_pool(name="psB", bufs=1, space="PSUM") as psB,
                tc.tile_pool(name="psC", bufs=1, space="PSUM") as psC,
            ):
                ps_chunks = {}

                def qkv_chunk(ch):
                    sl = slice(ch * 512, (ch + 1) * 512)
                    ps = [psA.tile([128, 512], f32, tag="qkv",
                                   name=f"qkv{o}c{ch}") for o in range(3)]
                    ps_chunks[ch] = ps
                    # two k-tiles per dma_start — the sync sequencer's
                    # per-trigger cost (~0.5us) would otherwise pace the
                    # x stream. Chunk 0 loads singles into SEPARATE tiles
                    # (per-tile dep tracking: a shared tile would make the
                    # first matmul wait for both halves) so the PE starts
                    # on k-tile 0 as soon as it lands.
                    for j in range(KT // 2):
                        if ch == 0:
                            xt = [xin.tile([128, 512], bf16, tag="xt1",
                                           name="xt1") for _ in range(2)]
                            for b in range(2):
                                nc.sync.dma_start(xt[b][:],
                                                  xT_p[j][:, b, sl])
                            rhs = [xt[0][:], xt[1][:]]
                        else:
                            xt = xin.tile([128, 2, 512], bf16, tag="xt",
                                          name="xt")
                            nc.sync.dma_start(xt[:], xT_p[j][:, :, sl])
                            rhs = [xt[:, 0, :], xt[:, 1, :]]
                        for b in range(2):
                            k = 2 * j + b
                            for o in range(3):
                                nc.tensor.matmul(
                                    ps[o][:],
                                    lhsT=wq_sb[k][:, o * 128:(o + 1) * 128],
                                    rhs=rhs[b],
                                    start=(k == 0), stop=(k == KT - 1))

                def finish_chunk(ch):
                    sl = slice(ch * 512, (ch + 1) * 512)
                    ps = ps_chunks.pop(ch)
                    nc.scalar.copy(qr_sb[0][:, sl], ps[0][:])
                    nc.scalar.copy(qr_sb[1][:, sl], ps[1][:])
                    # k duplicated into both partition halves so the flash
                    # score matmuls can contract 64 partitions at base 0
                    # (even heads) or base 64 (odd heads) — matmul requires
                    # lhsT/rhs base partitions to match
                    nc.scalar.copy(kr_sb[0:D, sl], ps[2][0:D, :])
                    nc.scalar.copy(kr_sb[D:128, sl], ps[2][0:D, :])
                    nc.scalar.copy(vT_sb[:, sl], ps[2][D:128, :])
                    # RoPE in place:  t = t*C + (R @ t)*S
                    for src_t in (qr_sb[0], qr_sb[1], kr_sb):
                        p = src_t.shape[0]
                        sw = psB.tile([128, 512], f32, tag="sw", name="sw")
                        nc.tensor.matmul(sw[:p, :], lhsT=rt_sb[:p, :p],
                                         rhs=src_t[:, sl],
                                         start=True, stop=True)
                        m1 = evp.tile([p, 512], bf16, tag="m1", name="m1")
                        nc.vector.tensor_mul(m1[:], src_t[:, sl],
                                             c_sb[:p, sl])
                        m2 = evp.tile([p, 512], bf16, tag="m2", name="m2")
                        nc.vector.tensor_mul(m2[:], sw[:p, :], s_sb[:p, sl])
                        nc.vector.tensor_add(src_t[:, sl], m1[:], m2[:])
                    # v transpose for this chunk's four st tiles, batched
                    # into one psum tile -> one DVE evacuation
                    pv = psC.tile([128, 4 * D], bf16, tag="vt", name="vt")
                    for j in range(4):
                        t = 4 * ch + j
                        nc.tensor.transpose(
                            pv[:, j * D:(j + 1) * D],
                            vT_sb[:, t * 128:(t + 1) * 128],
                            id_sb[:])
                    nc.vector.tensor_copy(
                        v_sb[:, 4 * ch:4 * ch + 4, 0:D],
                        pv[:].rearrange("p (t d) -> p t d", d=D))

                for ch in range(NC4 + 1):
                    if ch < NC4:
                        qkv_chunk(ch)
                    if ch >= 1:
                        finish_chunk(ch - 1)
                    # mask/wo loads ride the scalar queue, positioned so the
                    # triggers only fire after the 3rd/4th chunks' psum
                    # evacuations — their transfers then stay out of the
                    # x stream's HBM window (masks land ~10us before the
                    # first flash head, wo well before the first proj)
                    if ch == NC4 - 1:
                        for j in range(4):
                            nc.scalar.dma_start(mk_sb[j][:], mk_d[j])
                    if ch == NC4:
                        for i in range(2):
                            nc.scalar.dma_start(
                                wo_sb[i][:], wo_d[i * 128:(i + 1) * 128, :])

            # ---- phase D/E: flash attention with interleaved projection ----
            with (
                tc.tile_pool(name="scp", bufs=2, space="PSUM") as scp,
                tc.tile_pool(name="avp", bufs=2, space="PSUM") as avp,
                tc.tile_pool(name="mpp", bufs=2, space="PSUM") as mpp,
            ):
                def flash_head(c, h, uo_sb, l_sb):
                    csl = slice(c * 512, (c + 1) * 512)
                    n_st = 4 * c + 4
                    pair, half = divmod(h, 2)
                    qsl = slice(half * D, (half + 1) * D)
                    av = avp.tile([128, 512], f32, tag="av", name="av")

                    # diagonal tile t = 4c+j: the chunk's first 128j queries
                    # are fully masked — scores/mask/attn@V all run on the
                    # suffix only. The skipped sc prefix holds stale (but
                    # bounded) scores from a recycled psum buffer; their exp
                    # is finite and never read.
                    def toff(t):
                        return max(0, 128 * (t - 4 * c))

                    def av_group(g, pt):
                        for i in range(2):
                            t = 2 * g + i
                            off = toff(t)
                            psl = slice(i * 512 + off, (i + 1) * 512)
                            if t >= 4 * c:
                                nc.vector.tensor_mul(
                                    pt[:, psl], pt[:, psl],
                                    mk_sb[t - 4 * c][:, off:])
                            nc.tensor.matmul(
                                av[:D + 1, off:],
                                lhsT=v_sb[:, t, :],
                                rhs=pt[:, psl],
                                start=(t == 0), stop=(t == n_st - 1),
                                skip_group_check=True)

                    prev = None
                    for g in range(n_st // 2):
                        sc = scp.tile([128, 1024], f32, tag="sc", name="sc")
                        for i in range(2):
                            t = 2 * g + i
                            off = toff(t)
                            nc.tensor.matmul(
                                sc[:, i * 512 + off:(i + 1) * 512],
                                lhsT=kr_sb[qsl, t * 128:(t + 1) * 128],
                                rhs=qr_sb[pair][qsl,
                                                c * 512 + off:(c + 1) * 512],
                                start=True, stop=True)
                        pt = psb.tile([128, 1024], bf16, tag="P", name="P")
                        es = toff(2 * g)
                        nc.scalar.activation(
                            pt[:, es:], sc[:, es:],
                            mybir.ActivationFunctionType.Exp, scale=scale)
                        # emit the PREVIOUS group's attn@V after this group's
                        # scores so the PE never head-of-line blocks on exp
                        if prev is not None:
                            av_group(*prev)
                        prev = (g, pt)
                    av_group(*prev)
                    # stash unnormalized out^T and its denominator row
                    nc.vector.tensor_copy(uo_sb[:], av[0:D, :])
                    nc.vector.tensor_copy(l_sb[32 * h:32 * h + 1, :],
                                          av[D:D + 1, :])

                def norm_head(c, h, uo_sb, rcp):
                    csl = slice(c * 512, (c + 1) * 512)
                    pair, half = divmod(h, 2)
                    qsl = slice(half * D, (half + 1) * D)
                    # broadcast 1/l across the 64 head dims via a ones-column
                    # matmul (f32r rhs -> full-rate), then scale in one mul
                    bc = mpp.tile([128, 512], f32, tag="mp", name="mp")
                    nc.tensor.matmul(bc[:D, :], lhsT=on_sb[:],
                                     rhs=rcp[h][:], start=True, stop=True)
                    nc.vector.tensor_mul(
                        outT[pair][qsl, csl], uo_sb[:], bc[:D, :])

                def proj_group(c, hts, drain=False, queue=None):
                    csl = slice(c * 512, (c + 1) * 512)
                    ev = evo.tile([128, 4, 512], bf16, tag="ev", name="ev")
                    for n, ht in enumerate(hts):
                        pr = mpp.tile([128, 512], f32, tag="mp", name="mp")
                        for i in range(2):
                            nc.tensor.matmul(
                                pr[:],
                                lhsT=wo_sb[i][:, ht * 128:(ht + 1) * 128],
                                rhs=outT[i][:, csl],
                                start=(i == 0), stop=(i == 1))
                        # mid-flash the ScalarE is exp-bound: evacuate on
                        # DVE; in the drain both engines are free, split 2/2
                        if drain and ht % 2 == 1:
                            nc.scalar.copy(ev[:, n, :], pr[:])
                        else:
                            nc.vector.tensor_copy(ev[:, n, :], pr[:])
                    # one dma per 4 row-tiles (the sync sequencer's
                    # per-trigger cost would otherwise pace the drain)
                    (queue or nc.sync).dma_start(
                        out_p[:, hts[0]:hts[0] + 4, csl], ev[:])

                # pipeline: flash(ci) | normalize(ci-1) | project(ci-2) —
                # proj must trail normalization of ALL heads of its chunk.
                cs = [1, 2, 0, 3]
                uo_tiles = {}
                rcps = {}
                l_tiles = {}

                def rcp_chain(c):
                    # 1/l as exp(-ln l) on ScalarE: ln and exp share one act
                    # table, and it offloads the reciprocal from the busier
                    # DVE. Issued after the next chunk's first flash head so
                    # it never delays the boundary exp.
                    l_sb = l_tiles.pop(c)
                    lnl = evp.tile([128, 512], f32, tag="lnl", name="lnl")
                    nc.scalar.activation(
                        lnl[:97, :], l_sb[:97, :],
                        mybir.ActivationFunctionType.Ln)
                    rcp = evp.tile([128, 512], f32, tag="rcp", name="rcp")
                    nc.scalar.activation(
                        rcp[:97, :], lnl[:97, :],
                        mybir.ActivationFunctionType.Exp, scale=-1.0)
                    rows = []
                    for h in range(HPC):
                        rh = evp.tile([1, 512], bf16, tag=f"rch{h}",
                                      name=f"rch{h}")
                        nc.vector.tensor_copy(rh[:],
                                              rcp[32 * h:32 * h + 1, :])
                        rows.append(rh)
                    rcps[c] = rows

                for i in range(NC4):
                    l_sb = evp.tile([128, 512], f32, tag="l", name="l")
                    l_tiles[cs[i]] = l_sb
                    for h in range(HPC):
                        uo = evp.tile([D, 512], bf16, tag=f"uo{h}",
                                      name=f"uo{h}")
                        uo_tiles[(cs[i], h)] = uo
                        flash_head(cs[i], h, uo, l_sb)
                        # h==0: proj first so the rcp chain (ScalarE) has
                        # time to land behind the boundary exps; h>=1: norm
                        # first so the last norm mul is not queued behind
                        # this slot's proj evacuations on the DVE
                        if i >= 2 and h == 0:
                            proj_group(cs[i - 2], range(0, 4))
                        if i >= 1 and h == 0:
                            rcp_chain(cs[i - 1])
                        if i >= 1:
                            norm_head(cs[i - 1], h,
                                      uo_tiles.pop((cs[i - 1], h)),
                                      rcps[cs[i - 1]])
                        if i >= 2 and h >= 1:
                            proj_group(cs[i - 2], range(4 * h, 4 * h + 4))
                    if i == NC4 - 1:
                        # last chunk's reciprocal issued right behind its
                        # final exps so the drain's norms start immediately
                        rcp_chain(cs[i])

                # drain: start with the already-normalized chunk's proj (PE
                # filler that covers the last rcp chain's latency), then the
                # final chunk's norms, then the rest; out-writes alternate
                # between the sync and scalar DMA queues
                c_pen, c_fin = cs[NC4 - 2], cs[NC4 - 1]
                proj_group(c_pen, range(0, 4), drain=True, queue=nc.sync)
                proj_group(c_pen, range(4, 8), drain=True, queue=nc.scalar)
                for h in range(HPC):
                    norm_head(c_fin, h, uo_tiles.pop((c_fin, h)),
                              rcps[c_fin])
                proj_group(c_pen, range(8, 12), drain=True, queue=nc.sync)
                proj_group(c_pen, range(12, 16), drain=True, queue=nc.scalar)
                for g in range(HPC):
                    proj_group(c_fin, range(4 * g, 4 * g + 4), drain=True,
                               queue=(nc.sync if g % 2 == 0 else nc.scalar))

    _split_excess_waits(nc, mybir)
    return nc


def _host_prep(x, cos, sin, w_qkv, w_out):
    import ml_dtypes
    bf = ml_dtypes.bfloat16

    xT = np.ascontiguousarray(x[0].T).astype(bf)                # [H, S]
    cosT = cos.T.astype(np.float32)                             # [64, S]
    sinT = sin.T.astype(np.float32)
    ctile = np.ascontiguousarray(np.concatenate([cosT, cosT], 0)).astype(bf)
    stile = np.ascontiguousarray(np.concatenate([sinT, sinT], 0)).astype(bf)

    # rotate_half as a matrix: rot(q)^T = R @ q^T per 64-block; ship R^T
    r = np.zeros((D, D), dtype=np.float32)
    for i in range(32):
        r[i, 32 + i] = -1.0
        r[32 + i, i] = 1.0
    R = np.zeros((128, 128), dtype=np.float32)
    R[:D, :D] = r
    R[D:, D:] = r
    rotT = np.ascontiguousarray(R.T).astype(bf)

    ident = np.eye(D, dtype=np.float32).astype(bf)

    p = np.arange(128)[:, None]
    f = np.arange(512)[None, :]
    masks = np.stack([(p <= f - 128 * j).astype(bf) for j in range(4)])

    shared = {"xT": xT, "ctile": ctile, "stile": stile, "rotT": rotT,
              "ident": ident, "ones": np.ones((1, D), dtype=bf),
              "masks": masks}

    in_maps = []
    for c in range(N_CORES):
        qrows = w_qkv[4 * c * D:(4 * c + 4) * D]                # [256, H]
        krows = w_qkv[N_HEADS * D + c * D: N_HEADS * D + (c + 1) * D]
        vrows = w_qkv[(N_HEADS + N_KV_HEADS) * D + c * D:
                      (N_HEADS + N_KV_HEADS) * D + (c + 1) * D]
        wsh = np.concatenate([qrows, krows, vrows], 0)          # [384, H]
        wqkvT = np.ascontiguousarray(wsh.T).astype(bf)          # [H, 384]
        wo_cols = w_out[:, 4 * c * D:(4 * c + 4) * D]           # [H, 256]
        woutT = np.ascontiguousarray(wo_cols.T).astype(bf)
        in_maps.append({**shared, "wqkvT": wqkvT, "woutT": woutT})
    return in_maps


def kernel(x, cos, sin, w_qkv, w_out):
    from concourse.bass_utils import run_bass_kernel_spmd

    if "nc" not in _CACHE:
        _CACHE["nc"] = _build()
    nc = _CACHE["nc"]

    in_maps = _host_prep(x, cos, sin, w_qkv, w_out)
    res = run_bass_kernel_spmd(nc, in_maps, list(range(N_CORES)))
    total = np.zeros((HIDDEN, S), dtype=np.float32)
    for r in res.results:
        total += r["out"].astype(np.float32)
    return total.T.reshape(1, S, HIDDEN).copy()
